# revision 4
# baseline (speedup 1.0000x reference)
"""EvoBinarizedLayer as one fp8 matmul per population member.

Math: per population p, with xb = unpacked bits of x (LSB-first) and
w0/w1 the two unpacked weight bit-planes,

  count[p] = xb @ w0 + (1 - xb) @ w1
           = xb @ (w0 - w1) + colsum(w1)

so each core computes a single [512,2048] @ [2048,2048] matmul with
lhs entries in {0,1} and rhs entries in {-1,0,1} (both exact in fp8
e4m3, accumulated exactly in fp32 PSUM), plus a per-(p,o) bias added
on the host. Counts <= 2048 are exact in fp16, so the device emits
fp16 and the host upcasts to int32.

Sharding: population dim P=8, one member per NeuronCore (x replicated).

v2 schedule notes (from trace analysis of the 46.5us v1):
  - fp8 DoubleRow matmuls stream at 0.5 cycles/output-row, so an
    N=512 matmul is ~107ns; v1 was LDWEIGHTS-bound at ~216ns/matmul
    because every matmul reloaded the PE array. Here each LDWEIGHTS
    (one per (k-pair, batch-tile)) feeds TWO matmuls (two o-quarters,
    second with ldweights=False), dropping the PE floor to ~14us.
  - that makes the kernel DMA-bound: 5MB in + 2MB out on two HWDGE
    queues whose real limit is descriptor processing (~10-15ns/desc).
    DRAM layouts are per-partition-contiguous so k-half pieces give
    4KB descriptors (~2x the B/ns of v1's 2KB), with small 2KB-desc
    pieces only for the first k-quarter to start the stream early.
  - two passes over k: pass A accumulates o-quarters 0,1 in 8 PSUM
    banks (4 batch-tiles x 2), pass B o-quarters 2,3 in the same
    banks. Bank drains (CAST fp32->fp16) split across Vector and
    GpSimd so they hide under the opposite pass's matmul stream.
"""

import numpy as np
import ml_dtypes

POP, BATCH, IN_INTS, OUT_F = 8, 512, 32, 2048
K = IN_INTS * 64          # 2048 contraction (bit) dim
KT = K // 128             # 16 k-tiles of 128
N_CORES = 8
N_WARM = 20

_FP8 = ml_dtypes.float8_e4m3

_cached = {}


def _build_nc():
    import concourse.tile as tile
    from concourse import bacc, mybir

    dt = mybir.dt
    nc = bacc.Bacc(
        "TRN2", target_bir_lowering=False, debug=False, num_devices=N_CORES
    )
    xbt_d = nc.dram_tensor(
        "xbt", [128, KT, BATCH], dt.float8e4, kind="ExternalInput"
    ).ap()
    wd_d = nc.dram_tensor(
        "wd", [4, 128, KT, 512], dt.float8e4, kind="ExternalInput"
    ).ap()
    out_d = nc.dram_tensor(
        "out", [BATCH, OUT_F], dt.float16, kind="ExternalOutput"
    ).ap()

    with tile.TileContext(nc) as tc:
        with (
            tc.tile_pool(name="xbt", bufs=1) as xbt_pool,
            tc.tile_pool(name="wd", bufs=1) as wd_pool,
            tc.tile_pool(name="outp", bufs=8) as out_pool,
            tc.tile_pool(name="psum", bufs=8, space="PSUM") as psum_pool,
        ):
            S, A, G, V = nc.sync, nc.scalar, nc.gpsimd, nc.vector
            DR = mybir.MatmulPerfMode.DoubleRow

            # xbt_sb[p, k, b]: bit row k*128+p, batch b
            xbt_sb = xbt_pool.tile([128, KT, BATCH], dt.float8e4)
            # wd_sb[p, ob, k, o']: bit row k*128+p, out feature ob*512+o'
            wd_sb = wd_pool.tile([128, 4, KT, 512], dt.float8e4)

            # PE warmup: dummy DoubleRow matmuls on a small zeroed tile so
            # the HAM clock-gate opens before the real stream, bridging
            # until the first input pieces land.
            warm = xbt_pool.tile([128, 2, 128], dt.float8e4, tag="warm")
            nc.vector.memset(warm[:], 0.0)
            wps = psum_pool.tile([128, 512], dt.float32, tag="ps", name="warm_ps")
            for _ in range(N_WARM):
                nc.tensor.matmul(
                    wps[:, :128],
                    warm[:],
                    warm[:],
                    start=True,
                    stop=True,
                    perf_mode=DR,
                )

            # ---- input DMAs, strict need-order on the two HWDGE queues.
            # First k-quarter pieces are partition-split (64 x 2KB descs)
            # so the stream can start ~0.6us after queue spin-up; the rest
            # ride 2-4KB descriptors (descriptor processing is the queue
            # bottleneck, so bigger runs = more B/ns).
            S.dma_start(xbt_sb[0:64, 0:4, :], xbt_d[0:64, 0:4, :])
            A.dma_start(xbt_sb[64:128, 0:4, :], xbt_d[64:128, 0:4, :])
            S.dma_start(wd_sb[0:64, 0, 0:4, :], wd_d[0][0:64, 0:4, :])
            A.dma_start(wd_sb[64:128, 0, 0:4, :], wd_d[0][64:128, 0:4, :])
            S.dma_start(wd_sb[0:64, 1, 0:4, :], wd_d[1][0:64, 0:4, :])
            A.dma_start(wd_sb[64:128, 1, 0:4, :], wd_d[1][64:128, 0:4, :])
            # rest of pass A inputs, k-quarter then k-half granularity
            S.dma_start(xbt_sb[:, 4:8, :], xbt_d[:, 4:8, :])
            A.dma_start(wd_sb[:, 0, 4:8, :], wd_d[0][:, 4:8, :])
            S.dma_start(wd_sb[:, 1, 4:8, :], wd_d[1][:, 4:8, :])
            A.dma_start(xbt_sb[:, 8:16, :], xbt_d[:, 8:16, :])
            S.dma_start(wd_sb[:, 0, 8:16, :], wd_d[0][:, 8:16, :])
            A.dma_start(wd_sb[:, 1, 8:16, :], wd_d[1][:, 8:16, :])
            # pass B weights (4KB descs)
            S.dma_start(wd_sb[:, 2, 0:8, :], wd_d[2][:, 0:8, :])
            A.dma_start(wd_sb[:, 3, 0:8, :], wd_d[3][:, 0:8, :])
            S.dma_start(wd_sb[:, 2, 8:16, :], wd_d[2][:, 8:16, :])
            A.dma_start(wd_sb[:, 3, 8:16, :], wd_d[3][:, 8:16, :])

            # ---- PE stream: two passes of (8 k-pairs x 4 batch-tiles),
            # each LDWEIGHTS (x bits, K=256 DoubleRow) feeding two N=512
            # matmuls (the pass's two o-quarters; second skips the weight
            # reload). 8 PSUM banks = 4 bt x 2 ob per pass.
            def run_pass(oba, obb, ocol0):
                pss = [
                    [
                        psum_pool.tile(
                            [128, 512], dt.float32, tag="ps",
                            name=f"ps_{ocol0}_{bt}_{j}",
                        )
                        for j in range(2)
                    ]
                    for bt in range(4)
                ]
                stages = [
                    out_pool.tile(
                        [128, 1024], dt.float16, tag="ot",
                        name=f"ot_{ocol0}_{bt}",
                    )
                    for bt in range(4)
                ]
                for k in range(KT // 2):
                    last = k == KT // 2 - 1
                    for bt in range(4):
                        lhs = xbt_sb[:, 2 * k : 2 * k + 2, 128 * bt : 128 * (bt + 1)]
                        nc.tensor.matmul(
                            pss[bt][0][:],
                            lhs,
                            wd_sb[:, oba, 2 * k : 2 * k + 2, :],
                            start=(k == 0),
                            stop=last,
                            perf_mode=DR,
                        )
                        mm2 = nc.tensor.matmul(
                            pss[bt][1][:],
                            lhs,
                            wd_sb[:, obb, 2 * k : 2 * k + 2, :],
                            start=(k == 0),
                            stop=last,
                            perf_mode=DR,
                        )
                        mm2.ins.ldweights = False
                        if last:
                            # drain this bt's pair as soon as it stops so
                            # casts/out-DMAs stagger under remaining MMs.
                            # GpSimd has no PSUM port on TRN2, so split the
                            # casts between Vector and Activation (scalar).
                            V.tensor_copy(stages[bt][:, 0:512], pss[bt][0][:])
                            A.copy(stages[bt][:, 512:1024], pss[bt][1][:])
                            (A, S)[bt % 2].dma_start(
                                out_d[
                                    128 * bt : 128 * (bt + 1),
                                    ocol0 : ocol0 + 1024,
                                ],
                                stages[bt][:],
                            )

            run_pass(0, 1, 0)
            run_pass(2, 3, 1024)
    nc.compile()
    return nc


def get_nc():
    if "nc" not in _cached:
        _cached["nc"] = _build_nc()
    return _cached["nc"]


def pack_inputs(x, w):
    """Host-side bit unpack + layout. Returns (xbt, wd_cores, bias).

    xbt: [128, 16, BATCH] fp8; xbt[p, kt, b] = bit kt*128+p of x[b]
    wd_cores[p]: [4, 128, 16, 512] fp8; [ob, p, kt, o'] =
        (w0-w1) at bit row kt*128+p, out feature ob*512+o'
    bias: [POP, OUT_F] int32 colsum of w1 bits
    """
    xb = np.unpackbits(
        x.view(np.uint8).reshape(BATCH, IN_INTS, 8), axis=-1, bitorder="little"
    ).reshape(BATCH, K)
    xbt = np.ascontiguousarray(
        xb.T.reshape(KT, 128, BATCH).transpose(1, 0, 2)
    ).astype(_FP8)

    wbits = np.unpackbits(
        w.view(np.uint8).reshape(POP, IN_INTS, 2, OUT_F, 8),
        axis=-1,
        bitorder="little",
    )  # [POP, IN_INTS, 2, OUT_F, 64]
    w0 = wbits[:, :, 0].transpose(0, 1, 3, 2).reshape(POP, K, OUT_F)
    w1 = wbits[:, :, 1].transpose(0, 1, 3, 2).reshape(POP, K, OUT_F)
    bias = w1.sum(axis=1, dtype=np.int32)  # [POP, OUT_F]
    wd = w0.astype(np.int8) - w1.astype(np.int8)  # {-1,0,1}
    wd_cores = [
        np.ascontiguousarray(
            wd[p].reshape(KT, 128, 4, 512).transpose(2, 1, 0, 3)
        ).astype(_FP8)
        for p in range(POP)
    ]
    return xbt, wd_cores, bias


def kernel(x, w):
    from concourse.bass_utils import run_bass_kernel_spmd

    nc = get_nc()
    xbt, wd_cores, bias = pack_inputs(np.asarray(x), np.asarray(w))
    in_maps = [{"xbt": xbt, "wd": wd_cores[p]} for p in range(N_CORES)]
    try:
        res = run_bass_kernel_spmd(nc, in_maps, list(range(N_CORES)))
    except Exception:
        # NRT_EXEC_UNIT_UNRECOVERABLE has been observed transiently on this
        # fabric; one retry has always succeeded.
        res = run_bass_kernel_spmd(nc, in_maps, list(range(N_CORES)))
    out = np.empty((POP, BATCH, OUT_F), dtype=np.int32)
    for p in range(N_CORES):
        out[p] = res.results[p]["out"].astype(np.int32) + bias[p][None, :]
    return out


# revision 5
# speedup vs baseline: 1.0224x; 1.0224x over previous
"""EvoBinarizedLayer as one fp8 matmul per population member.

Math: per population p, with xb = unpacked bits of x (LSB-first) and
w0/w1 the two unpacked weight bit-planes,

  count[p] = xb @ w0 + (1 - xb) @ w1
           = xb @ (w0 - w1) + colsum(w1)

so each core computes a single [512,2048] @ [2048,2048] matmul with
lhs entries in {0,1} and rhs entries in {-1,0,1} (both exact in fp8
e4m3, accumulated exactly in fp32 PSUM), plus a per-(p,o) bias added
on the host. Counts <= 2048 are exact in fp16, so the device emits
fp16 and the host upcasts to int32.

Sharding: population dim P=8, one member per NeuronCore (x replicated).

v2 schedule notes (from trace analysis of the 46.5us v1):
  - fp8 DoubleRow matmuls stream at 0.5 cycles/output-row, so an
    N=512 matmul is ~107ns; v1 was LDWEIGHTS-bound at ~216ns/matmul
    because every matmul reloaded the PE array. Here each LDWEIGHTS
    (one per (k-pair, batch-tile)) feeds TWO matmuls (two o-quarters,
    second with ldweights=False), dropping the PE floor to ~14us.
  - that makes the kernel DMA-bound: 5MB in + 2MB out on two HWDGE
    queues whose real limit is descriptor processing (~10-15ns/desc).
    DRAM layouts are per-partition-contiguous so k-half pieces give
    4KB descriptors (~2x the B/ns of v1's 2KB), with small 2KB-desc
    pieces only for the first k-quarter to start the stream early.
  - two passes over k: pass A accumulates o-quarters 0,1 in 8 PSUM
    banks (4 batch-tiles x 2), pass B o-quarters 2,3 in the same
    banks. Bank drains (CAST fp32->fp16) split across Vector and
    GpSimd so they hide under the opposite pass's matmul stream.
"""

import numpy as np
import ml_dtypes

POP, BATCH, IN_INTS, OUT_F = 8, 512, 32, 2048
K = IN_INTS * 64          # 2048 contraction (bit) dim
KT = K // 128             # 16 k-tiles of 128
N_CORES = 8
N_WARM = 20

_FP8 = ml_dtypes.float8_e4m3

_cached = {}


def _build_nc():
    import concourse.tile as tile
    from concourse import bacc, mybir

    dt = mybir.dt
    nc = bacc.Bacc(
        "TRN2", target_bir_lowering=False, debug=False, num_devices=N_CORES
    )
    xbt_d = nc.dram_tensor(
        "xbt", [128, KT, BATCH], dt.float8e4, kind="ExternalInput"
    ).ap()
    wd_d = nc.dram_tensor(
        "wd", [4, 128, KT, 512], dt.float8e4, kind="ExternalInput"
    ).ap()
    out_d = nc.dram_tensor(
        "out", [BATCH, OUT_F], dt.float16, kind="ExternalOutput"
    ).ap()

    with tile.TileContext(nc) as tc:
        with (
            tc.tile_pool(name="xbt", bufs=1) as xbt_pool,
            tc.tile_pool(name="wd", bufs=1) as wd_pool,
            tc.tile_pool(name="outp", bufs=8) as out_pool,
            tc.tile_pool(name="psum", bufs=8, space="PSUM") as psum_pool,
        ):
            S, A, G, V = nc.sync, nc.scalar, nc.gpsimd, nc.vector
            DR = mybir.MatmulPerfMode.DoubleRow

            # xbt_sb[p, k, b]: bit row k*128+p, batch b
            xbt_sb = xbt_pool.tile([128, KT, BATCH], dt.float8e4)
            # wd_sb[p, ob, k, o']: bit row k*128+p, out feature ob*512+o'
            wd_sb = wd_pool.tile([128, 4, KT, 512], dt.float8e4)

            # PE warmup: dummy DoubleRow matmuls on a small zeroed tile so
            # the HAM clock-gate opens before the real stream, bridging
            # until the first input pieces land.
            warm = xbt_pool.tile([128, 2, 128], dt.float8e4, tag="warm")
            nc.vector.memset(warm[:], 0.0)
            wps = psum_pool.tile([128, 512], dt.float32, tag="ps", name="warm_ps")
            for _ in range(N_WARM):
                nc.tensor.matmul(
                    wps[:, :128],
                    warm[:],
                    warm[:],
                    start=True,
                    stop=True,
                    perf_mode=DR,
                )

            # ---- input DMAs, strict need-order on the two HWDGE queues.
            # First k-quarter pieces are partition-split (64 x 2KB descs)
            # so the stream can start ~0.6us after queue spin-up; the rest
            # ride 2-4KB descriptors (descriptor processing is the queue
            # bottleneck, so bigger runs = more B/ns).
            S.dma_start(xbt_sb[0:64, 0:4, :], xbt_d[0:64, 0:4, :])
            A.dma_start(xbt_sb[64:128, 0:4, :], xbt_d[64:128, 0:4, :])
            S.dma_start(wd_sb[0:64, 0, 0:4, :], wd_d[0][0:64, 0:4, :])
            A.dma_start(wd_sb[64:128, 0, 0:4, :], wd_d[0][64:128, 0:4, :])
            S.dma_start(wd_sb[0:64, 1, 0:4, :], wd_d[1][0:64, 0:4, :])
            A.dma_start(wd_sb[64:128, 1, 0:4, :], wd_d[1][64:128, 0:4, :])
            # rest of pass A inputs, k-quarter then k-half granularity
            S.dma_start(xbt_sb[:, 4:8, :], xbt_d[:, 4:8, :])
            A.dma_start(wd_sb[:, 0, 4:8, :], wd_d[0][:, 4:8, :])
            S.dma_start(wd_sb[:, 1, 4:8, :], wd_d[1][:, 4:8, :])
            A.dma_start(xbt_sb[:, 8:16, :], xbt_d[:, 8:16, :])
            S.dma_start(wd_sb[:, 0, 8:16, :], wd_d[0][:, 8:16, :])
            A.dma_start(wd_sb[:, 1, 8:16, :], wd_d[1][:, 8:16, :])
            # pass B weights (4KB descs)
            S.dma_start(wd_sb[:, 2, 0:8, :], wd_d[2][:, 0:8, :])
            A.dma_start(wd_sb[:, 3, 0:8, :], wd_d[3][:, 0:8, :])
            S.dma_start(wd_sb[:, 2, 8:16, :], wd_d[2][:, 8:16, :])
            A.dma_start(wd_sb[:, 3, 8:16, :], wd_d[3][:, 8:16, :])

            # ---- PE stream: two passes of (8 k-pairs x 4 batch-tiles),
            # each LDWEIGHTS (x bits, K=256 DoubleRow) feeding two N=512
            # matmuls (the pass's two o-quarters; second skips the weight
            # reload). 8 PSUM banks = 4 bt x 2 ob per pass.
            def run_pass(oba, obb, ocol0):
                pss = [
                    [
                        psum_pool.tile(
                            [128, 512], dt.float32, tag="ps",
                            name=f"ps_{ocol0}_{bt}_{j}",
                        )
                        for j in range(2)
                    ]
                    for bt in range(4)
                ]
                stages = [
                    out_pool.tile(
                        [128, 1024], dt.float16, tag="ot",
                        name=f"ot_{ocol0}_{bt}",
                    )
                    for bt in range(4)
                ]
                for k in range(KT // 2):
                    last = k == KT // 2 - 1
                    for bt in range(4):
                        lhs = xbt_sb[:, 2 * k : 2 * k + 2, 128 * bt : 128 * (bt + 1)]
                        nc.tensor.matmul(
                            pss[bt][0][:],
                            lhs,
                            wd_sb[:, oba, 2 * k : 2 * k + 2, :],
                            start=(k == 0),
                            stop=last,
                            perf_mode=DR,
                        )
                        mm2 = nc.tensor.matmul(
                            pss[bt][1][:],
                            lhs,
                            wd_sb[:, obb, 2 * k : 2 * k + 2, :],
                            start=(k == 0),
                            stop=last,
                            perf_mode=DR,
                        )
                        mm2.ins.ldweights = False
                        if last:
                            # drain this bt's pair as soon as it stops so
                            # casts/out-DMAs stagger under remaining MMs.
                            # GpSimd has no PSUM port on TRN2, so split the
                            # casts between Vector and Activation (scalar).
                            V.tensor_copy(stages[bt][:, 0:512], pss[bt][0][:])
                            A.copy(stages[bt][:, 512:1024], pss[bt][1][:])
                            (A, S)[bt % 2].dma_start(
                                out_d[
                                    128 * bt : 128 * (bt + 1),
                                    ocol0 : ocol0 + 1024,
                                ],
                                stages[bt][:],
                            )

            run_pass(0, 1, 0)
            run_pass(2, 3, 1024)

    # The tile lowering emits one InstLdweights per matmul even when
    # consecutive matmuls share the same stationary operand. Drop the
    # redundant reloads (the PE weight registers hold until the next
    # LDWEIGHTS): this is what lets each (k, bt) x-tile feed both
    # o-quarter matmuls and halves the PE stream's critical rate.
    # Freshly inserted LDWs carry no deps/descendants (the matmuls own
    # them), so removal is safe; matmul waits then attach to the
    # surviving LDW of the pair, which only strengthens ordering.
    for f in nc.m.functions:
        for bb in f.blocks:
            prev_sig = None
            drop = []
            for i in bb.instructions:
                if type(i).__name__ == "InstLdweights":
                    sig = (
                        str(i.ins[0]),
                        str(i.perf_mode),
                        str(i.tile_position),
                        str(i.is_transpose),
                    )
                    if sig == prev_sig:
                        drop.append(i)
                    prev_sig = sig
            for i in drop:
                bb.instructions.remove(i)

    nc.compile()
    return nc


def get_nc():
    if "nc" not in _cached:
        _cached["nc"] = _build_nc()
    return _cached["nc"]


def pack_inputs(x, w):
    """Host-side bit unpack + layout. Returns (xbt, wd_cores, bias).

    xbt: [128, 16, BATCH] fp8; xbt[p, kt, b] = bit kt*128+p of x[b]
    wd_cores[p]: [4, 128, 16, 512] fp8; [ob, p, kt, o'] =
        (w0-w1) at bit row kt*128+p, out feature ob*512+o'
    bias: [POP, OUT_F] int32 colsum of w1 bits
    """
    xb = np.unpackbits(
        x.view(np.uint8).reshape(BATCH, IN_INTS, 8), axis=-1, bitorder="little"
    ).reshape(BATCH, K)
    xbt = np.ascontiguousarray(
        xb.T.reshape(KT, 128, BATCH).transpose(1, 0, 2)
    ).astype(_FP8)

    wbits = np.unpackbits(
        w.view(np.uint8).reshape(POP, IN_INTS, 2, OUT_F, 8),
        axis=-1,
        bitorder="little",
    )  # [POP, IN_INTS, 2, OUT_F, 64]
    w0 = wbits[:, :, 0].transpose(0, 1, 3, 2).reshape(POP, K, OUT_F)
    w1 = wbits[:, :, 1].transpose(0, 1, 3, 2).reshape(POP, K, OUT_F)
    bias = w1.sum(axis=1, dtype=np.int32)  # [POP, OUT_F]
    wd = w0.astype(np.int8) - w1.astype(np.int8)  # {-1,0,1}
    wd_cores = [
        np.ascontiguousarray(
            wd[p].reshape(KT, 128, 4, 512).transpose(2, 1, 0, 3)
        ).astype(_FP8)
        for p in range(POP)
    ]
    return xbt, wd_cores, bias


def kernel(x, w):
    from concourse.bass_utils import run_bass_kernel_spmd

    nc = get_nc()
    xbt, wd_cores, bias = pack_inputs(np.asarray(x), np.asarray(w))
    in_maps = [{"xbt": xbt, "wd": wd_cores[p]} for p in range(N_CORES)]
    try:
        res = run_bass_kernel_spmd(nc, in_maps, list(range(N_CORES)))
    except Exception:
        # NRT_EXEC_UNIT_UNRECOVERABLE has been observed transiently on this
        # fabric; one retry has always succeeded.
        res = run_bass_kernel_spmd(nc, in_maps, list(range(N_CORES)))
    out = np.empty((POP, BATCH, OUT_F), dtype=np.int32)
    for p in range(N_CORES):
        out[p] = res.results[p]["out"].astype(np.int32) + bias[p][None, :]
    return out
